# revision 39
# baseline (speedup 1.0000x reference)
"""Trainium2 Bass kernel for DifferentiableExtrusion.

Full inputs in, full output out. Sharding: the 96x96=9216 grid points are
split across 8 cores (12 grid rows / 1152 points each). Every core processes
all valid polygons (host-compacted) against its points.

Per (point, edge), with affine-in-point quantities computed on the PE from
bf16 hi/lo-split features (K=32):
    u  = v.e / sqrt(e^2+eps)          (proj coordinate)
    uS = u - S                        (S = sqrt(e^2+eps))
    l  = v x e / |e|                  (line distance)
    Gt = "gated" ray-cast term: Gt = G = inter_x - px on rows the edge
         straddles; Gt = x0 - px - BIG < 0 on rows it doesn't (the one-hot
         row features carry a per-(row,edge) hi/lo-split offset that cancels
         the slope term and subtracts BIG).
Then:
    b   = relu(uS)                    (ACT)
    r   = min(u,0) + b                (DVE stt; == u - clip(u,0,S))
    d^2 = l^2 + r^2                   (ACT squares -> GpSimd add, bf16)
    mind2 = min over 32 edges         (DVE reduce)
    prod  = PRODUCT of Gt over edges  (DVE mult-reduce; its SIGN BIT is the
            crossing parity: inside <=> odd #positives <=> odd #negatives)
    q   = mind2 with prod's sign bit  (2 bitwise ops)
Final: per-batch min over polys; sigmoid(-100*sign*sqrt(|q|)) computed with
exp/ln only (sqrt z = exp(ln(z)/2), sigmoid = 1/(1+e^x) via DVE reciprocal)
so the ENTIRE kernel uses one ACT table set (natural_log_exp_and_others).
Depth extrusion = DRAM-bounce row broadcast as before.
"""

import numpy as np

VOX = 96
SHARP = 100.0
EPS = 1e-8
NCORES = 8
M = VOX * VOX
MP = M // NCORES          # 1152 points per core
CHUNKS = MP // 128        # 9
ROWS = MP // VOX          # 12 grid rows per core
PEDGES = 32               # edges per polygon
BIGD = 1e3                # far-outside distance for dummy (empty-batch) polys
BIGG = 8.0                # Gt offset for non-straddling rows
KF = 32                   # feature rows: 8 affine-split + 12 row-hi + 12 row-lo

MM_DTYPE = "bfloat16"


def _b16split(x):
    import ml_dtypes
    hi = x.astype(ml_dtypes.bfloat16).astype(np.float64)
    lo = (x - hi).astype(ml_dtypes.bfloat16).astype(np.float64)
    return hi, lo


def _expand_w(w):
    # value = hi + lo with bf16-exact components; features are
    # [hx, lx, hx, hy, ly, hy, 1, 1] so rows per coeff i<2 are
    # [w_hi (vs hx), w_hi (vs lx), w_lo (vs hx)]; bias uses [hi, lo].
    out = np.zeros((8, w.shape[1]), np.float64)
    for i in range(3):
        hi, lo = _b16split(w[i])
        if i < 2:
            j = i * 3
            out[j] = hi; out[j + 1] = hi; out[j + 2] = lo
        else:
            out[6] = hi; out[7] = lo
    return out


def _host_prep(polygons, attributes, validity_scores):
    import ml_dtypes

    B, N, P, _ = polygons.shape
    assert P == PEDGES
    valid = np.asarray(validity_scores) >= 0.5
    counts = [max(1, int(v.sum())) for v in valid]
    offs = np.cumsum([0] + counts)
    NPT = int(offs[-1])
    E = NPT * P

    v0 = np.asarray(polygons, np.float32).astype(np.float64)
    v1 = np.roll(v0, -1, axis=2)
    x0, y0 = v0[..., 0], v0[..., 1]
    x1, y1 = v1[..., 0], v1[..., 1]
    ex, ey = x1 - x0, y1 - y0
    esq = ex * ex + ey * ey
    esq_c = np.maximum(esq, 1e-12)
    Sp = np.sqrt(esq + EPS)
    rt = np.sqrt(esq_c)
    s = ex / (ey + EPS)

    cu = np.stack([ex / Sp, ey / Sp, -(x0 * ex + y0 * ey) / Sp], -1)
    cuS = cu.copy()
    cuS[..., 2] -= Sp
    cl = np.stack([-ey / rt, ex / rt, (ey * x0 - ex * y0) / rt], -1)
    cg = np.stack([-np.ones_like(s), s, x0 - s * y0], -1)

    # compact per-batch valid polys into column tables [3, E]
    def compact(c):
        w = np.zeros((3, E), np.float64)
        for b in range(B):
            idx = np.nonzero(valid[b])[0]
            for k, n in enumerate(idx):
                c0 = (offs[b] + k) * P
                w[:, c0:c0 + P] = c[b, n].T
        return w

    wu3, wuS3, wl3, wg3 = compact(cu), compact(cuS), compact(cl), compact(cg)
    y0c = np.full(E, 5.0, np.float64)
    y1c = np.full(E, 5.0, np.float64)
    sc = np.zeros(E, np.float64)
    for b in range(B):
        idx = np.nonzero(valid[b])[0]
        for k, n in enumerate(idx):
            c0 = (offs[b] + k) * P
            y0c[c0:c0 + P] = y0[b, n]
            y1c[c0:c0 + P] = y1[b, n]
            sc[c0:c0 + P] = s[b, n]

    # dummy cols (empty batches): u = uS = 0, l = BIGD, Gt = -1
    dummy = np.ones(E, bool)
    for b in range(B):
        if valid[b].any():
            dummy[offs[b] * P:offs[b + 1] * P] = False
    wl3[:, dummy] = 0.0
    wl3[2, dummy] = BIGD
    wg3[:, dummy] = 0.0
    wg3[2, dummy] = -1.0

    Wu = _expand_w(wu3)
    WuS = _expand_w(wuS3)
    Wl = _expand_w(wl3)
    Wg = _expand_w(wg3)

    # Per-core weight tables [KF, 4E] and feature tiles [CHUNKS, KF, 128]
    ygrid, xgrid = np.meshgrid(np.arange(VOX, dtype=np.float64),
                               np.arange(VOX, dtype=np.float64), indexing="ij")
    px = (xgrid.ravel() / np.float64(VOX - 1))
    py = (ygrid.ravel() / np.float64(VOX - 1))

    # half-permutation for the u/uS/l streams: edges 0-15 of every poly in
    # the first E/2 columns, edges 16-31 in the second — lets GpSimd do a
    # first-level min (pairing col i with col i+E/2) before the DVE reduce.
    hperm = np.concatenate([
        (np.arange(NPT)[:, None] * P + np.arange(P // 2)[None, :]).ravel(),
        (np.arange(NPT)[:, None] * P + np.arange(P // 2, P)[None, :]).ravel()])

    Ws, feats = [], []
    for k in range(NCORES):
        rows_y = (np.arange(ROWS, dtype=np.float64) + k * ROWS) / (VOX - 1)
        t0c = (y0c[None, :] <= rows_y[:, None])
        t1c = (y1c[None, :] <= rows_y[:, None])
        ysq = (t0c ^ t1c)                                  # [ROWS, E]
        # row-offset for Gt: 0 where straddling, else -(kappa + BIGG) with
        # kappa = s*(y_r - y0) the slope term it cancels.
        kappa = sc[None, :] * (rows_y[:, None] - y0c[None, :])
        delta = np.where(ysq, 0.0, -(kappa + BIGG))        # [ROWS, E]
        delta[:, dummy] = 0.0
        dhi, dlo = _b16split(delta)

        W = np.zeros((KF, 4 * E), np.float64)
        W[:8, 0 * E:1 * E] = Wu[:, hperm]
        W[:8, 1 * E:2 * E] = WuS[:, hperm]
        W[:8, 2 * E:3 * E] = Wl[:, hperm]
        W[:8, 3 * E:4 * E] = Wg
        W[8:8 + ROWS, 3 * E:4 * E] = dhi
        W[8 + ROWS:8 + 2 * ROWS, 3 * E:4 * E] = dlo
        Ws.append(np.ascontiguousarray(
            W.astype(np.float32).astype(ml_dtypes.bfloat16)))

        sl = slice(k * MP, (k + 1) * MP)
        hx, lx = _b16split(px[sl])
        hy, ly = _b16split(py[sl])
        one = np.ones(MP)
        f = np.stack([hx, lx, hx, hy, ly, hy, one, one], 0)   # [8, MP]
        rowid = (np.arange(MP) // VOX)                        # local row
        oneh = (rowid[None, :] == np.arange(ROWS)[:, None]).astype(np.float64)
        F = np.concatenate([f, oneh, oneh], 0)                # [KF, MP]
        feats.append(np.ascontiguousarray(
            F.astype(np.float32).astype(ml_dtypes.bfloat16)))

    attr = np.asarray(attributes, np.float32)
    norm_h = np.clip(attr[:, 0], 0.0, 1.0)
    hv = np.clip(np.round(norm_h * VOX), 1.0, float(VOX)).astype(np.float32)
    hvs = [0 if not valid[b].any() else int(hv[b]) for b in range(B)]

    tables = {"ident": np.eye(128, dtype=np.float32)}
    return tables, Ws, feats, counts, E, hvs


def _build(B, counts, E, hvs):
    import concourse.tile as tile
    from concourse import bacc, mybir

    f32 = mybir.dt.float32
    i32 = mybir.dt.int32
    bf16 = mybir.dt.bfloat16
    mmdt = getattr(mybir.dt, MM_DTYPE)

    Op = mybir.AluOpType
    Act = mybir.ActivationFunctionType
    X = mybir.AxisListType.X
    NPT = sum(counts)
    offs = np.cumsum([0] + list(counts))
    NB = 5                     # blocks per chunk
    nb = E // NB               # 416 cols, 13 polys
    assert nb * NB == E and nb % PEDGES == 0
    npb = nb // PEDGES         # polys per block
    SIGN_MASK = -2147483648    # 0x80000000
    ONE_BITS = 1065353216      # 0x3F800000 (float 1.0)

    nc = bacc.Bacc("TRN2", target_bir_lowering=False, debug=False)

    din = {
        "W": nc.dram_tensor("W", [KF, 4 * E], mmdt, kind="ExternalInput"),
        "feat": nc.dram_tensor("feat", [KF, MP], mmdt,
                               kind="ExternalInput"),
        "ident": nc.dram_tensor("ident", [128, 128], f32,
                                kind="ExternalInput"),
    }
    out_d = nc.dram_tensor("out", [B, MP], f32, kind="ExternalOutput")

    with tile.TileContext(nc) as tc:
        with tc.tile_pool(name="const", bufs=1) as cpool, \
             tc.tile_pool(name="work", bufs=2) as wpool, \
             tc.tile_pool(name="acc", bufs=2) as apool, \
             tc.tile_pool(name="ps_u", bufs=2, space="PSUM") as pu, \
             tc.tile_pool(name="ps_us", bufs=2, space="PSUM") as pus, \
             tc.tile_pool(name="ps_l", bufs=1, space="PSUM") as pl, \
             tc.tile_pool(name="ps_g", bufs=2, space="PSUM") as pg, \
             tc.tile_pool(name="pout", bufs=1, space="PSUM") as opool:

            # constants in; W split into many DMAs so the queues fill in
            # parallel (one or two queues would gate the first matmuls)
            # keep ALL start-up DMA dispatch off the scalar queue so the ACT
            # table load issues immediately
            Wsb = cpool.tile([KF, 4 * E], mmdt)
            wengs = [nc.sync, nc.gpsimd]
            NSPL = 4
            step = -(-4 * E // NSPL)
            for i in range(NSPL):
                c0 = i * step
                c1 = min(4 * E, c0 + step)
                if c0 >= c1:
                    break
                wengs[i % 2].dma_start(Wsb[:, c0:c1], din["W"][:, c0:c1])
            featsb = cpool.tile([KF, CHUNKS * 128], mmdt)
            nc.sync.dma_start(featsb[:], din["feat"][:])
            ident = cpool.tile([128, 128], f32)
            nc.gpsimd.dma_start(ident[:], din["ident"][:])

            qbig = cpool.tile([128, CHUNKS, NPT], f32)

            # force the single ACT table set (natural_log_exp_and_others)
            # to load once, up front: Ln is its anchor function.
            warm = cpool.tile([1, 1], f32)
            nc.gpsimd.memset(warm[:], 1.0)
            nc.scalar.activation(warm[:], warm[:], Act.Ln)

            for c in range(CHUNKS):
                featc = featsb[:, c * 128:(c + 1) * 128]
                mind2 = apool.tile([128, NPT], f32, tag="mind2")
                gprod = apool.tile([128, NPT], f32, tag="gprod")
                lsq = apool.tile([128, E], bf16, tag="lsq")
                rall = apool.tile([128, E], bf16, tag="rall")
                for j in range(NB):
                    cs = slice(j * nb, (j + 1) * nb)
                    U = pu.tile([128, nb], f32, tag="u")
                    US = pus.tile([128, nb], f32, tag="us")
                    L = pl.tile([128, nb], f32, tag="l")
                    GT = pg.tile([128, nb], f32, tag="g")
                    nc.tensor.matmul(US[:], featc, Wsb[:, 1 * E:][:, cs])
                    nc.tensor.matmul(L[:], featc, Wsb[:, 2 * E:][:, cs])
                    nc.tensor.matmul(U[:], featc, Wsb[:, 0 * E:][:, cs])
                    nc.tensor.matmul(GT[:], featc, Wsb[:, 3 * E:][:, cs])

                    b_ = wpool.tile([128, nb], bf16, tag="b")
                    nc.scalar.activation(b_[:], US[:], Act.Relu)
                    nc.scalar.activation(lsq[:, cs], L[:], Act.Square)
                    nc.vector.scalar_tensor_tensor(
                        rall[:, cs], U[:], 0.0, b_[:], op0=Op.min, op1=Op.add)
                    ps = slice(j * npb, (j + 1) * npb)
                    with nc.allow_low_precision(
                            reason="only the product's sign bit is used"):
                        nc.vector.tensor_reduce(
                            gprod[:, ps],
                            GT[:].rearrange("p (a b) -> p a b", b=PEDGES),
                            axis=X, op=Op.mult)

                # chunk-wide: r^2 (split between ACT and GpSimd to balance
                # engine load); d2 = l^2 + r^2 on GpSimd; first-level min on
                # the DVE (u/uS/l columns are half-permuted so col i pairs
                # with col i + E/2), then the final 16-wide min reduce.
                rsq = apool.tile([128, E], bf16, tag="rsq")
                nc.scalar.activation(rsq[:, :E // 2], rall[:, :E // 2],
                                     Act.Square)
                nc.gpsimd.tensor_tensor(rsq[:, E // 2:], rall[:, E // 2:],
                                        rall[:, E // 2:], op=Op.mult)
                d2 = apool.tile([128, E], bf16, tag="d2")
                nc.gpsimd.tensor_tensor(d2[:], lsq[:], rsq[:], op=Op.add)
                m16 = apool.tile([128, E // 2], bf16, tag="m16")
                nc.vector.tensor_tensor(m16[:], d2[:, :E // 2], d2[:, E // 2:],
                                        op=Op.min)
                nc.vector.tensor_reduce(
                    mind2[:],
                    m16[:].rearrange("p (a b) -> p a b", b=PEDGES // 2),
                    axis=X, op=Op.min)

                # q = mind2 * sign(gprod): gprod's sign is the crossing parity
                sgn = wpool.tile([128, NPT], f32, tag="sgn")
                nc.scalar.activation(sgn[:], gprod[:], Act.Sign)
                nc.vector.tensor_tensor(qbig[:, c, :], mind2[:], sgn[:],
                                        op=Op.mult)

            # per-batch min over polys -> qall[:, 32b + c]
            qall = cpool.tile([128, B * 32], f32)
            nc.gpsimd.memset(qall[:], 0)
            for b in range(B):
                nc.vector.tensor_reduce(
                    qall[:, 32 * b:32 * b + CHUNKS],
                    qbig[:, :, offs[b]:offs[b + 1]], axis=X, op=Op.min)

            # end stage: sigmoid(-SHARP * sign(q) * sqrt(|q|)) via exp/ln:
            #   rt = exp(ln(|q|)/2) = sqrt(|q|), capped at 0.5 (=> |x|<=50)
            #   out = 1 / (1 + exp(SHARP * sign(q) * rt))
            absq = wpool.tile([128, B * 32], f32, tag="absq")
            nc.scalar.activation(absq[:], qall[:], Act.Abs)
            absq2 = wpool.tile([128, B * 32], f32, tag="absq2")
            nc.vector.tensor_scalar(absq2[:], absq[:], 1e-30, None, op0=Op.add)
            lnq = wpool.tile([128, B * 32], f32, tag="lnq")
            nc.scalar.activation(lnq[:], absq2[:], Act.Ln)
            rt = wpool.tile([128, B * 32], f32, tag="rt")
            nc.scalar.activation(rt[:], lnq[:], Act.Exp, scale=0.5)
            sgq = wpool.tile([128, B * 32], f32, tag="sgq")
            nc.scalar.activation(sgq[:], qall[:], Act.Sign)
            w_ = wpool.tile([128, B * 32], f32, tag="w")
            nc.vector.scalar_tensor_tensor(
                w_[:], rt[:], 0.5, sgq[:], op0=Op.min, op1=Op.mult)
            ex_ = wpool.tile([128, B * 32], f32, tag="ex")
            nc.scalar.activation(ex_[:], w_[:], Act.Exp, scale=SHARP)
            s1 = wpool.tile([128, B * 32], f32, tag="s1")
            nc.vector.tensor_scalar(s1[:], ex_[:], 1.0, None, op0=Op.add)
            cpb = wpool.tile([128, B * 32], f32, tag="cpb")
            nc.vector.reciprocal(cpb[:], s1[:])

            pst = opool.tile([128, 128], f32, tag="pp", name="pst")
            nc.tensor.transpose(pst[:], cpb[:], ident[:])
            comb = cpool.tile([128, 128], f32, tag="comb", name="comb")
            nc.scalar.activation(comb[:], pst[:], Act.Copy)

            # the device output is the [B, MP] soft mask; the host replicates
            # it across the depth axis (pure data tiling) during unshard.
            engs = [nc.sync, nc.gpsimd, nc.scalar]
            for b in range(B):
                engs[b % 3].dma_start(out_d[b:b + 1, :],
                                      comb[32 * b:32 * b + CHUNKS, :])

    nc.compile()
    return nc


def _enable_ldw_opt():
    # no-op: --enable-ldw-opt=true breaks walrus codegen on the transpose's
    # standalone InstLdweights (kept so callers can still invoke it).
    return


def kernel(polygons, attributes, validity_scores):
    from concourse.bass_utils import run_bass_kernel_spmd

    B = polygons.shape[0]
    tables, Ws, feats, counts, E, hvs = _host_prep(
        polygons, attributes, validity_scores)
    nc = _build(B, counts, E, hvs)
    in_maps = [dict(tables, W=Ws[k], feat=feats[k]) for k in range(NCORES)]
    res = run_bass_kernel_spmd(nc, in_maps, list(range(NCORES))).results
    parts = [res[k]["out"].reshape(B, VOX // NCORES, VOX)
             for k in range(NCORES)]
    combined = np.concatenate(parts, axis=1)          # [B, VOX, VOX]
    dmask = (np.arange(VOX, dtype=np.float32)[None, :]
             < np.array(hvs, np.float32)[:, None]).astype(np.float32)
    voxels = combined[:, None, :, :] * dmask[:, :, None, None]
    return np.ascontiguousarray(voxels, np.float32)


# revision 40
# speedup vs baseline: 1.0045x; 1.0045x over previous
"""Trainium2 Bass kernel for DifferentiableExtrusion.

Full inputs in, full output out. Sharding: the 96x96=9216 grid points are
split across 8 cores (12 grid rows / 1152 points each). Every core processes
all valid polygons (host-compacted) against its points.

Per (point, edge), with affine-in-point quantities computed on the PE from
bf16 hi/lo-split features (K=32):
    u  = v.e / sqrt(e^2+eps)          (proj coordinate)
    uS = u - S                        (S = sqrt(e^2+eps))
    l  = v x e / |e|                  (line distance)
    Gt = "gated" ray-cast term: Gt = G = inter_x - px on rows the edge
         straddles; Gt = x0 - px - BIG < 0 on rows it doesn't (the one-hot
         row features carry a per-(row,edge) hi/lo-split offset that cancels
         the slope term and subtracts BIG).
Then:
    b   = relu(uS)                    (ACT)
    r   = min(u,0) + b                (DVE stt; == u - clip(u,0,S))
    d^2 = l^2 + r^2                   (ACT squares -> GpSimd add, bf16)
    mind2 = min over 32 edges         (DVE reduce)
    prod  = PRODUCT of Gt over edges  (DVE mult-reduce; its SIGN BIT is the
            crossing parity: inside <=> odd #positives <=> odd #negatives)
    q   = mind2 with prod's sign bit  (2 bitwise ops)
Final: per-batch min over polys; sigmoid(-100*sign*sqrt(|q|)) computed with
exp/ln only (sqrt z = exp(ln(z)/2), sigmoid = 1/(1+e^x) via DVE reciprocal)
so the ENTIRE kernel uses one ACT table set (natural_log_exp_and_others).
Depth extrusion = DRAM-bounce row broadcast as before.
"""

import numpy as np

VOX = 96
SHARP = 100.0
EPS = 1e-8
NCORES = 8
M = VOX * VOX
MP = M // NCORES          # 1152 points per core
CHUNKS = MP // 128        # 9
ROWS = MP // VOX          # 12 grid rows per core
PEDGES = 32               # edges per polygon
BIGD = 1e3                # far-outside distance for dummy (empty-batch) polys
BIGG = 8.0                # Gt offset for non-straddling rows
KF = 32                   # feature rows: 8 affine-split + 12 row-hi + 12 row-lo

MM_DTYPE = "bfloat16"


def _b16split(x):
    import ml_dtypes
    hi = x.astype(ml_dtypes.bfloat16).astype(np.float64)
    lo = (x - hi).astype(ml_dtypes.bfloat16).astype(np.float64)
    return hi, lo


def _expand_w(w):
    # value = hi + lo with bf16-exact components; features are
    # [hx, lx, hx, hy, ly, hy, 1, 1] so rows per coeff i<2 are
    # [w_hi (vs hx), w_hi (vs lx), w_lo (vs hx)]; bias uses [hi, lo].
    out = np.zeros((8, w.shape[1]), np.float64)
    for i in range(3):
        hi, lo = _b16split(w[i])
        if i < 2:
            j = i * 3
            out[j] = hi; out[j + 1] = hi; out[j + 2] = lo
        else:
            out[6] = hi; out[7] = lo
    return out


def _host_prep(polygons, attributes, validity_scores):
    import ml_dtypes

    B, N, P, _ = polygons.shape
    assert P == PEDGES
    valid = np.asarray(validity_scores) >= 0.5
    counts = [max(1, int(v.sum())) for v in valid]
    offs = np.cumsum([0] + counts)
    NPT = int(offs[-1])
    E = NPT * P

    v0 = np.asarray(polygons, np.float32).astype(np.float64)
    v1 = np.roll(v0, -1, axis=2)
    x0, y0 = v0[..., 0], v0[..., 1]
    x1, y1 = v1[..., 0], v1[..., 1]
    ex, ey = x1 - x0, y1 - y0
    esq = ex * ex + ey * ey
    esq_c = np.maximum(esq, 1e-12)
    Sp = np.sqrt(esq + EPS)
    rt = np.sqrt(esq_c)
    s = ex / (ey + EPS)

    cu = np.stack([ex / Sp, ey / Sp, -(x0 * ex + y0 * ey) / Sp], -1)
    cuS = cu.copy()
    cuS[..., 2] -= Sp
    cl = np.stack([-ey / rt, ex / rt, (ey * x0 - ex * y0) / rt], -1)
    cg = np.stack([-np.ones_like(s), s, x0 - s * y0], -1)

    # compact per-batch valid polys into column tables [3, E]
    def compact(c):
        w = np.zeros((3, E), np.float64)
        for b in range(B):
            idx = np.nonzero(valid[b])[0]
            for k, n in enumerate(idx):
                c0 = (offs[b] + k) * P
                w[:, c0:c0 + P] = c[b, n].T
        return w

    wu3, wuS3, wl3, wg3 = compact(cu), compact(cuS), compact(cl), compact(cg)
    y0c = np.full(E, 5.0, np.float64)
    y1c = np.full(E, 5.0, np.float64)
    sc = np.zeros(E, np.float64)
    for b in range(B):
        idx = np.nonzero(valid[b])[0]
        for k, n in enumerate(idx):
            c0 = (offs[b] + k) * P
            y0c[c0:c0 + P] = y0[b, n]
            y1c[c0:c0 + P] = y1[b, n]
            sc[c0:c0 + P] = s[b, n]

    # dummy cols (empty batches): u = uS = 0, l = BIGD, Gt = -1
    dummy = np.ones(E, bool)
    for b in range(B):
        if valid[b].any():
            dummy[offs[b] * P:offs[b + 1] * P] = False
    wl3[:, dummy] = 0.0
    wl3[2, dummy] = BIGD
    wg3[:, dummy] = 0.0
    wg3[2, dummy] = -1.0

    Wu = _expand_w(wu3)
    WuS = _expand_w(wuS3)
    Wl = _expand_w(wl3)
    Wg = _expand_w(wg3)

    # Per-core weight tables [KF, 4E] and feature tiles [CHUNKS, KF, 128]
    ygrid, xgrid = np.meshgrid(np.arange(VOX, dtype=np.float64),
                               np.arange(VOX, dtype=np.float64), indexing="ij")
    px = (xgrid.ravel() / np.float64(VOX - 1))
    py = (ygrid.ravel() / np.float64(VOX - 1))

    # half-permutation for the u/uS/l streams: edges 0-15 of every poly in
    # the first E/2 columns, edges 16-31 in the second — lets GpSimd do a
    # first-level min (pairing col i with col i+E/2) before the DVE reduce.
    hperm = np.concatenate([
        (np.arange(NPT)[:, None] * P + np.arange(P // 2)[None, :]).ravel(),
        (np.arange(NPT)[:, None] * P + np.arange(P // 2, P)[None, :]).ravel()])

    Ws, feats = [], []
    for k in range(NCORES):
        rows_y = (np.arange(ROWS, dtype=np.float64) + k * ROWS) / (VOX - 1)
        t0c = (y0c[None, :] <= rows_y[:, None])
        t1c = (y1c[None, :] <= rows_y[:, None])
        ysq = (t0c ^ t1c)                                  # [ROWS, E]
        # row-offset for Gt: 0 where straddling, else -(kappa + BIGG) with
        # kappa = s*(y_r - y0) the slope term it cancels.
        kappa = sc[None, :] * (rows_y[:, None] - y0c[None, :])
        delta = np.where(ysq, 0.0, -(kappa + BIGG))        # [ROWS, E]
        delta[:, dummy] = 0.0
        dhi, dlo = _b16split(delta)

        W = np.zeros((KF, 4 * E), np.float64)
        W[:8, 0 * E:1 * E] = Wu[:, hperm]
        W[:8, 1 * E:2 * E] = WuS[:, hperm]
        W[:8, 2 * E:3 * E] = Wl[:, hperm]
        W[:8, 3 * E:4 * E] = Wg
        W[8:8 + ROWS, 3 * E:4 * E] = dhi
        W[8 + ROWS:8 + 2 * ROWS, 3 * E:4 * E] = dlo
        Ws.append(np.ascontiguousarray(
            W.astype(np.float32).astype(ml_dtypes.bfloat16)))

        sl = slice(k * MP, (k + 1) * MP)
        hx, lx = _b16split(px[sl])
        hy, ly = _b16split(py[sl])
        one = np.ones(MP)
        f = np.stack([hx, lx, hx, hy, ly, hy, one, one], 0)   # [8, MP]
        rowid = (np.arange(MP) // VOX)                        # local row
        oneh = (rowid[None, :] == np.arange(ROWS)[:, None]).astype(np.float64)
        F = np.concatenate([f, oneh, oneh], 0)                # [KF, MP]
        feats.append(np.ascontiguousarray(
            F.astype(np.float32).astype(ml_dtypes.bfloat16)))

    attr = np.asarray(attributes, np.float32)
    norm_h = np.clip(attr[:, 0], 0.0, 1.0)
    hv = np.clip(np.round(norm_h * VOX), 1.0, float(VOX)).astype(np.float32)
    hvs = [0 if not valid[b].any() else int(hv[b]) for b in range(B)]

    tables = {"ident": np.eye(128, dtype=np.float32)}
    return tables, Ws, feats, counts, E, hvs


def _build(B, counts, E, hvs):
    import concourse.tile as tile
    from concourse import bacc, mybir

    f32 = mybir.dt.float32
    i32 = mybir.dt.int32
    bf16 = mybir.dt.bfloat16
    mmdt = getattr(mybir.dt, MM_DTYPE)

    Op = mybir.AluOpType
    Act = mybir.ActivationFunctionType
    X = mybir.AxisListType.X
    NPT = sum(counts)
    offs = np.cumsum([0] + list(counts))
    NB = 5                     # blocks per chunk
    nb = E // NB               # 416 cols, 13 polys
    assert nb * NB == E and nb % PEDGES == 0
    npb = nb // PEDGES         # polys per block
    SIGN_MASK = -2147483648    # 0x80000000
    ONE_BITS = 1065353216      # 0x3F800000 (float 1.0)

    nc = bacc.Bacc("TRN2", target_bir_lowering=False, debug=False)

    din = {
        "W": nc.dram_tensor("W", [KF, 4 * E], mmdt, kind="ExternalInput"),
        "feat": nc.dram_tensor("feat", [KF, MP], mmdt,
                               kind="ExternalInput"),
        "ident": nc.dram_tensor("ident", [128, 128], f32,
                                kind="ExternalInput"),
    }
    out_d = nc.dram_tensor("out", [B, MP], f32, kind="ExternalOutput")

    with tile.TileContext(nc) as tc:
        with tc.tile_pool(name="const", bufs=1) as cpool, \
             tc.tile_pool(name="work", bufs=2) as wpool, \
             tc.tile_pool(name="acc", bufs=2) as apool, \
             tc.tile_pool(name="ps_u", bufs=2, space="PSUM") as pu, \
             tc.tile_pool(name="ps_us", bufs=2, space="PSUM") as pus, \
             tc.tile_pool(name="ps_l", bufs=1, space="PSUM") as pl, \
             tc.tile_pool(name="ps_g", bufs=2, space="PSUM") as pg, \
             tc.tile_pool(name="pout", bufs=1, space="PSUM") as opool:

            # constants in; W split into many DMAs so the queues fill in
            # parallel (one or two queues would gate the first matmuls)
            # keep ALL start-up DMA dispatch off the scalar queue so the ACT
            # table load issues immediately
            Wsb = cpool.tile([KF, 4 * E], mmdt)
            wengs = [nc.sync, nc.gpsimd]
            NSPL = 4
            step = -(-4 * E // NSPL)
            for i in range(NSPL):
                c0 = i * step
                c1 = min(4 * E, c0 + step)
                if c0 >= c1:
                    break
                wengs[i % 2].dma_start(Wsb[:, c0:c1], din["W"][:, c0:c1])
            featsb = cpool.tile([KF, CHUNKS * 128], mmdt)
            nc.sync.dma_start(featsb[:], din["feat"][:])
            ident = cpool.tile([128, 128], f32)
            nc.gpsimd.dma_start(ident[:], din["ident"][:])

            qbig = cpool.tile([128, CHUNKS, NPT], f32)

            # force the single ACT table set (natural_log_exp_and_others)
            # to load once, up front: Ln is its anchor function.
            warm = cpool.tile([1, 1], f32)
            nc.gpsimd.memset(warm[:], 1.0)
            nc.scalar.activation(warm[:], warm[:], Act.Ln)

            for c in range(CHUNKS):
                featc = featsb[:, c * 128:(c + 1) * 128]
                mind2 = apool.tile([128, NPT], f32, tag="mind2")
                gprod = apool.tile([128, NPT], f32, tag="gprod")
                lsq = apool.tile([128, E], bf16, tag="lsq")
                rall = apool.tile([128, E], bf16, tag="rall")
                for j in range(NB):
                    cs = slice(j * nb, (j + 1) * nb)
                    U = pu.tile([128, nb], f32, tag="u")
                    US = pus.tile([128, nb], f32, tag="us")
                    L = pl.tile([128, nb], f32, tag="l")
                    GT = pg.tile([128, nb], f32, tag="g")
                    nc.tensor.matmul(US[:], featc, Wsb[:, 1 * E:][:, cs])
                    nc.tensor.matmul(L[:], featc, Wsb[:, 2 * E:][:, cs])
                    nc.tensor.matmul(U[:], featc, Wsb[:, 0 * E:][:, cs])
                    nc.tensor.matmul(GT[:], featc, Wsb[:, 3 * E:][:, cs])

                    b_ = wpool.tile([128, nb], bf16, tag="b")
                    nc.scalar.activation(b_[:], US[:], Act.Relu)
                    nc.scalar.activation(lsq[:, cs], L[:], Act.Square)
                    nc.vector.scalar_tensor_tensor(
                        rall[:, cs], U[:], 0.0, b_[:], op0=Op.min, op1=Op.add)
                    ps = slice(j * npb, (j + 1) * npb)
                    with nc.allow_low_precision(
                            reason="only the product's sign bit is used"):
                        nc.vector.tensor_reduce(
                            gprod[:, ps],
                            GT[:].rearrange("p (a b) -> p a b", b=PEDGES),
                            axis=X, op=Op.mult)

                # chunk-wide: r^2 (split between ACT and GpSimd to balance
                # engine load); d2 = l^2 + r^2 on GpSimd; first-level min on
                # the DVE (u/uS/l columns are half-permuted so col i pairs
                # with col i + E/2), then the final 16-wide min reduce.
                rsq = apool.tile([128, E], bf16, tag="rsq")
                nc.scalar.activation(rsq[:, :E // 2], rall[:, :E // 2],
                                     Act.Square)
                nc.gpsimd.tensor_tensor(rsq[:, E // 2:], rall[:, E // 2:],
                                        rall[:, E // 2:], op=Op.mult)
                d2 = apool.tile([128, E], bf16, tag="d2")
                nc.vector.tensor_tensor(d2[:, :E // 2], lsq[:, :E // 2],
                                        rsq[:, :E // 2], op=Op.add)
                nc.gpsimd.tensor_tensor(d2[:, E // 2:], lsq[:, E // 2:],
                                        rsq[:, E // 2:], op=Op.add)
                m16 = apool.tile([128, E // 2], bf16, tag="m16")
                nc.vector.tensor_tensor(m16[:], d2[:, :E // 2], d2[:, E // 2:],
                                        op=Op.min)
                nc.vector.tensor_reduce(
                    mind2[:],
                    m16[:].rearrange("p (a b) -> p a b", b=PEDGES // 2),
                    axis=X, op=Op.min)

                # q = mind2 * sign(gprod): gprod's sign is the crossing parity
                sgn = wpool.tile([128, NPT], f32, tag="sgn")
                nc.scalar.activation(sgn[:], gprod[:], Act.Sign)
                nc.vector.tensor_tensor(qbig[:, c, :], mind2[:], sgn[:],
                                        op=Op.mult)

            # per-batch min over polys -> qall[:, 32b + c]
            qall = cpool.tile([128, B * 32], f32)
            nc.gpsimd.memset(qall[:], 0)
            for b in range(B):
                nc.vector.tensor_reduce(
                    qall[:, 32 * b:32 * b + CHUNKS],
                    qbig[:, :, offs[b]:offs[b + 1]], axis=X, op=Op.min)

            # end stage: sigmoid(-SHARP * sign(q) * sqrt(|q|)) via exp/ln:
            #   rt = exp(ln(|q|)/2) = sqrt(|q|), capped at 0.5 (=> |x|<=50)
            #   out = 1 / (1 + exp(SHARP * sign(q) * rt))
            absq = wpool.tile([128, B * 32], f32, tag="absq")
            nc.scalar.activation(absq[:], qall[:], Act.Abs)
            absq2 = wpool.tile([128, B * 32], f32, tag="absq2")
            nc.vector.tensor_scalar(absq2[:], absq[:], 1e-30, None, op0=Op.add)
            lnq = wpool.tile([128, B * 32], f32, tag="lnq")
            nc.scalar.activation(lnq[:], absq2[:], Act.Ln)
            rt = wpool.tile([128, B * 32], f32, tag="rt")
            nc.scalar.activation(rt[:], lnq[:], Act.Exp, scale=0.5)
            sgq = wpool.tile([128, B * 32], f32, tag="sgq")
            nc.scalar.activation(sgq[:], qall[:], Act.Sign)
            w_ = wpool.tile([128, B * 32], f32, tag="w")
            nc.vector.scalar_tensor_tensor(
                w_[:], rt[:], 0.5, sgq[:], op0=Op.min, op1=Op.mult)
            ex_ = wpool.tile([128, B * 32], f32, tag="ex")
            nc.scalar.activation(ex_[:], w_[:], Act.Exp, scale=SHARP)
            s1 = wpool.tile([128, B * 32], f32, tag="s1")
            nc.vector.tensor_scalar(s1[:], ex_[:], 1.0, None, op0=Op.add)
            cpb = wpool.tile([128, B * 32], f32, tag="cpb")
            nc.vector.reciprocal(cpb[:], s1[:])

            pst = opool.tile([128, 128], f32, tag="pp", name="pst")
            nc.tensor.transpose(pst[:], cpb[:], ident[:])
            comb = cpool.tile([128, 128], f32, tag="comb", name="comb")
            nc.scalar.activation(comb[:], pst[:], Act.Copy)

            # the device output is the [B, MP] soft mask; the host replicates
            # it across the depth axis (pure data tiling) during unshard.
            engs = [nc.sync, nc.gpsimd, nc.scalar]
            for b in range(B):
                engs[b % 3].dma_start(out_d[b:b + 1, :],
                                      comb[32 * b:32 * b + CHUNKS, :])

    nc.compile()
    return nc


def _enable_ldw_opt():
    # no-op: --enable-ldw-opt=true breaks walrus codegen on the transpose's
    # standalone InstLdweights (kept so callers can still invoke it).
    return


def kernel(polygons, attributes, validity_scores):
    from concourse.bass_utils import run_bass_kernel_spmd

    B = polygons.shape[0]
    tables, Ws, feats, counts, E, hvs = _host_prep(
        polygons, attributes, validity_scores)
    nc = _build(B, counts, E, hvs)
    in_maps = [dict(tables, W=Ws[k], feat=feats[k]) for k in range(NCORES)]
    res = run_bass_kernel_spmd(nc, in_maps, list(range(NCORES))).results
    parts = [res[k]["out"].reshape(B, VOX // NCORES, VOX)
             for k in range(NCORES)]
    combined = np.concatenate(parts, axis=1)          # [B, VOX, VOX]
    dmask = (np.arange(VOX, dtype=np.float32)[None, :]
             < np.array(hvs, np.float32)[:, None]).astype(np.float32)
    voxels = combined[:, None, :, :] * dmask[:, :, None, None]
    return np.ascontiguousarray(voxels, np.float32)


# revision 43
# speedup vs baseline: 1.0434x; 1.0387x over previous
"""Trainium2 Bass kernel for DifferentiableExtrusion.

Full inputs in, full output out. Sharding: the 96x96=9216 grid points are
split across 8 cores (12 grid rows / 1152 points each). Every core processes
all valid polygons (host-compacted) against its points.

Per (point, edge), with affine-in-point quantities computed on the PE from
bf16 hi/lo-split features (K=32):
    u  = v.e / sqrt(e^2+eps)          (proj coordinate)
    uS = u - S                        (S = sqrt(e^2+eps))
    l  = v x e / |e|                  (line distance)
    Gt = "gated" ray-cast term: Gt = G = inter_x - px on rows the edge
         straddles; Gt = x0 - px - BIG < 0 on rows it doesn't (the one-hot
         row features carry a per-(row,edge) hi/lo-split offset that cancels
         the slope term and subtracts BIG).
Then:
    b   = relu(uS)                    (ACT)
    r   = min(u,0) + b                (DVE stt; == u - clip(u,0,S))
    d^2 = l^2 + r^2                   (ACT squares -> GpSimd add, bf16)
    mind2 = min over 32 edges         (DVE reduce)
    prod  = PRODUCT of Gt over edges  (DVE mult-reduce; its SIGN BIT is the
            crossing parity: inside <=> odd #positives <=> odd #negatives)
    q   = mind2 with prod's sign bit  (2 bitwise ops)
Final: per-batch min over polys; sigmoid(-100*sign*sqrt(|q|)) computed with
exp/ln only (sqrt z = exp(ln(z)/2), sigmoid = 1/(1+e^x) via DVE reciprocal)
so the ENTIRE kernel uses one ACT table set (natural_log_exp_and_others).
Depth extrusion = DRAM-bounce row broadcast as before.
"""

import numpy as np

VOX = 96
SHARP = 100.0
EPS = 1e-8
NCORES = 8
M = VOX * VOX
MP = M // NCORES          # 1152 points per core
CHUNKS = MP // 128        # 9
ROWS = MP // VOX          # 12 grid rows per core
PEDGES = 32               # edges per polygon
BIGD = 1e3                # far-outside distance for dummy (empty-batch) polys
BIGG = 8.0                # Gt offset for non-straddling rows
KF = 32                   # feature rows: 8 affine-split + 12 row-hi + 12 row-lo

MM_DTYPE = "bfloat16"


def _b16split(x):
    import ml_dtypes
    hi = x.astype(ml_dtypes.bfloat16).astype(np.float64)
    lo = (x - hi).astype(ml_dtypes.bfloat16).astype(np.float64)
    return hi, lo


def _expand_w(w):
    # value = hi + lo with bf16-exact components; features are
    # [hx, lx, hx, hy, ly, hy, 1, 1] so rows per coeff i<2 are
    # [w_hi (vs hx), w_hi (vs lx), w_lo (vs hx)]; bias uses [hi, lo].
    out = np.zeros((8, w.shape[1]), np.float64)
    for i in range(3):
        hi, lo = _b16split(w[i])
        if i < 2:
            j = i * 3
            out[j] = hi; out[j + 1] = hi; out[j + 2] = lo
        else:
            out[6] = hi; out[7] = lo
    return out


def _host_prep(polygons, attributes, validity_scores):
    import ml_dtypes

    B, N, P, _ = polygons.shape
    assert P == PEDGES
    valid = np.asarray(validity_scores) >= 0.5
    counts = [max(1, int(v.sum())) for v in valid]
    offs = np.cumsum([0] + counts)
    NPT = int(offs[-1])
    E = NPT * P

    v0 = np.asarray(polygons, np.float32).astype(np.float64)
    v1 = np.roll(v0, -1, axis=2)
    x0, y0 = v0[..., 0], v0[..., 1]
    x1, y1 = v1[..., 0], v1[..., 1]
    ex, ey = x1 - x0, y1 - y0
    esq = ex * ex + ey * ey
    esq_c = np.maximum(esq, 1e-12)
    Sp = np.sqrt(esq + EPS)
    rt = np.sqrt(esq_c)
    s = ex / (ey + EPS)

    cu = np.stack([ex / Sp, ey / Sp, -(x0 * ex + y0 * ey) / Sp], -1)
    cuS = cu.copy()
    cuS[..., 2] -= Sp
    cl = np.stack([-ey / rt, ex / rt, (ey * x0 - ex * y0) / rt], -1)
    cg = np.stack([-np.ones_like(s), s, x0 - s * y0], -1)

    # compact per-batch valid polys into column tables [3, E]
    def compact(c):
        w = np.zeros((3, E), np.float64)
        for b in range(B):
            idx = np.nonzero(valid[b])[0]
            for k, n in enumerate(idx):
                c0 = (offs[b] + k) * P
                w[:, c0:c0 + P] = c[b, n].T
        return w

    wu3, wuS3, wl3, wg3 = compact(cu), compact(cuS), compact(cl), compact(cg)
    y0c = np.full(E, 5.0, np.float64)
    y1c = np.full(E, 5.0, np.float64)
    sc = np.zeros(E, np.float64)
    for b in range(B):
        idx = np.nonzero(valid[b])[0]
        for k, n in enumerate(idx):
            c0 = (offs[b] + k) * P
            y0c[c0:c0 + P] = y0[b, n]
            y1c[c0:c0 + P] = y1[b, n]
            sc[c0:c0 + P] = s[b, n]

    # dummy cols (empty batches): u = uS = 0, l = BIGD, Gt = -1
    dummy = np.ones(E, bool)
    for b in range(B):
        if valid[b].any():
            dummy[offs[b] * P:offs[b + 1] * P] = False
    wl3[:, dummy] = 0.0
    wl3[2, dummy] = BIGD
    wg3[:, dummy] = 0.0
    wg3[2, dummy] = -1.0

    Wu = _expand_w(wu3)
    WuS = _expand_w(wuS3)
    Wl = _expand_w(wl3)
    Wg = _expand_w(wg3)

    # Per-core weight tables [KF, 4E] and feature tiles [CHUNKS, KF, 128]
    ygrid, xgrid = np.meshgrid(np.arange(VOX, dtype=np.float64),
                               np.arange(VOX, dtype=np.float64), indexing="ij")
    px = (xgrid.ravel() / np.float64(VOX - 1))
    py = (ygrid.ravel() / np.float64(VOX - 1))

    # half-permutation for the u/uS/l streams: edges 0-15 of every poly in
    # the first E/2 columns, edges 16-31 in the second — lets GpSimd do a
    # first-level min (pairing col i with col i+E/2) before the DVE reduce.
    hperm = np.concatenate([
        (np.arange(NPT)[:, None] * P + np.arange(P // 2)[None, :]).ravel(),
        (np.arange(NPT)[:, None] * P + np.arange(P // 2, P)[None, :]).ravel()])

    Ws, feats = [], []
    for k in range(NCORES):
        rows_y = (np.arange(ROWS, dtype=np.float64) + k * ROWS) / (VOX - 1)
        t0c = (y0c[None, :] <= rows_y[:, None])
        t1c = (y1c[None, :] <= rows_y[:, None])
        ysq = (t0c ^ t1c)                                  # [ROWS, E]
        # row-offset for Gt: 0 where straddling, else -(kappa + BIGG) with
        # kappa = s*(y_r - y0) the slope term it cancels.
        kappa = sc[None, :] * (rows_y[:, None] - y0c[None, :])
        delta = np.where(ysq, 0.0, -(kappa + BIGG))        # [ROWS, E]
        delta[:, dummy] = 0.0
        dhi, dlo = _b16split(delta)

        W = np.zeros((KF, 4 * E), np.float64)
        W[:8, 0 * E:1 * E] = Wu[:, hperm]
        W[:8, 1 * E:2 * E] = WuS[:, hperm]
        W[:8, 2 * E:3 * E] = Wl[:, hperm]
        W[:8, 3 * E:4 * E] = Wg
        W[8:8 + ROWS, 3 * E:4 * E] = dhi
        W[8 + ROWS:8 + 2 * ROWS, 3 * E:4 * E] = dlo
        Ws.append(np.ascontiguousarray(
            W.astype(np.float32).astype(ml_dtypes.bfloat16)))

        sl = slice(k * MP, (k + 1) * MP)
        hx, lx = _b16split(px[sl])
        hy, ly = _b16split(py[sl])
        one = np.ones(MP)
        f = np.stack([hx, lx, hx, hy, ly, hy, one, one], 0)   # [8, MP]
        rowid = (np.arange(MP) // VOX)                        # local row
        oneh = (rowid[None, :] == np.arange(ROWS)[:, None]).astype(np.float64)
        F = np.concatenate([f, oneh, oneh], 0)                # [KF, MP]
        feats.append(np.ascontiguousarray(
            F.astype(np.float32).astype(ml_dtypes.bfloat16)))

    attr = np.asarray(attributes, np.float32)
    norm_h = np.clip(attr[:, 0], 0.0, 1.0)
    hv = np.clip(np.round(norm_h * VOX), 1.0, float(VOX)).astype(np.float32)
    hvs = [0 if not valid[b].any() else int(hv[b]) for b in range(B)]

    tables = {"ident": np.eye(128, dtype=np.float32)}
    return tables, Ws, feats, counts, E, hvs


def _build(B, counts, E, hvs):
    import concourse.tile as tile
    from concourse import bacc, mybir

    f32 = mybir.dt.float32
    i32 = mybir.dt.int32
    bf16 = mybir.dt.bfloat16
    mmdt = getattr(mybir.dt, MM_DTYPE)

    Op = mybir.AluOpType
    Act = mybir.ActivationFunctionType
    X = mybir.AxisListType.X
    NPT = sum(counts)
    offs = np.cumsum([0] + list(counts))
    NB = 5                     # blocks per chunk
    nb = E // NB               # 416 cols, 13 polys
    assert nb * NB == E and nb % PEDGES == 0
    npb = nb // PEDGES         # polys per block
    SIGN_MASK = -2147483648    # 0x80000000
    ONE_BITS = 1065353216      # 0x3F800000 (float 1.0)

    nc = bacc.Bacc("TRN2", target_bir_lowering=False, debug=False)

    din = {
        "W": nc.dram_tensor("W", [KF, 4 * E], mmdt, kind="ExternalInput"),
        "feat": nc.dram_tensor("feat", [KF, MP], mmdt,
                               kind="ExternalInput"),
        "ident": nc.dram_tensor("ident", [128, 128], f32,
                                kind="ExternalInput"),
    }
    out_d = nc.dram_tensor("out", [B, MP], f32, kind="ExternalOutput")

    with tile.TileContext(nc) as tc:
        with tc.tile_pool(name="const", bufs=1) as cpool, \
             tc.tile_pool(name="work", bufs=2) as wpool, \
             tc.tile_pool(name="acc", bufs=2) as apool, \
             tc.tile_pool(name="ps_u", bufs=2, space="PSUM") as pu, \
             tc.tile_pool(name="ps_us", bufs=2, space="PSUM") as pus, \
             tc.tile_pool(name="ps_l", bufs=1, space="PSUM") as pl, \
             tc.tile_pool(name="ps_g", bufs=2, space="PSUM") as pg, \
             tc.tile_pool(name="pout", bufs=1, space="PSUM") as opool:

            # constants in; W split into many DMAs so the queues fill in
            # parallel (one or two queues would gate the first matmuls)
            # keep ALL start-up DMA dispatch off the scalar queue so the ACT
            # table load issues immediately
            Wsb = cpool.tile([KF, 4 * E], mmdt)
            wengs = [nc.sync, nc.gpsimd]
            NSPL = 4
            step = -(-4 * E // NSPL)
            for i in range(NSPL):
                c0 = i * step
                c1 = min(4 * E, c0 + step)
                if c0 >= c1:
                    break
                wengs[i % 2].dma_start(Wsb[:, c0:c1], din["W"][:, c0:c1])
            featsb = cpool.tile([KF, CHUNKS * 128], mmdt)
            nc.sync.dma_start(featsb[:], din["feat"][:])
            ident = cpool.tile([128, 128], f32)
            nc.gpsimd.dma_start(ident[:], din["ident"][:])

            mindbig = cpool.tile([128, CHUNKS, NPT], f32)
            gprodbig = cpool.tile([128, CHUNKS, NPT], f32)
            qbig = cpool.tile([128, CHUNKS, NPT], f32)

            # force the single ACT table set (natural_log_exp_and_others)
            # to load once, up front: Ln is its anchor function.
            warm = cpool.tile([1, 1], f32)
            nc.gpsimd.memset(warm[:], 1.0)
            nc.scalar.activation(warm[:], warm[:], Act.Ln)

            for c in range(CHUNKS):
                featc = featsb[:, c * 128:(c + 1) * 128]
                mind2 = mindbig[:, c, :]
                gprod = gprodbig[:, c, :]
                lsq = apool.tile([128, E], bf16, tag="lsq")
                rall = apool.tile([128, E], bf16, tag="rall")
                for j in range(NB):
                    cs = slice(j * nb, (j + 1) * nb)
                    U = pu.tile([128, nb], f32, tag="u")
                    US = pus.tile([128, nb], f32, tag="us")
                    L = pl.tile([128, nb], f32, tag="l")
                    GT = pg.tile([128, nb], f32, tag="g")
                    nc.tensor.matmul(US[:], featc, Wsb[:, 1 * E:][:, cs])
                    nc.tensor.matmul(L[:], featc, Wsb[:, 2 * E:][:, cs])
                    nc.tensor.matmul(U[:], featc, Wsb[:, 0 * E:][:, cs])
                    nc.tensor.matmul(GT[:], featc, Wsb[:, 3 * E:][:, cs])

                    b_ = wpool.tile([128, nb], bf16, tag="b")
                    nc.scalar.activation(b_[:], US[:], Act.Relu)
                    nc.scalar.activation(lsq[:, cs], L[:], Act.Square)
                    nc.vector.scalar_tensor_tensor(
                        rall[:, cs], U[:], 0.0, b_[:], op0=Op.min, op1=Op.add)
                    ps = slice(j * npb, (j + 1) * npb)
                    with nc.allow_low_precision(
                            reason="only the product's sign bit is used"):
                        nc.vector.tensor_reduce(
                            gprod[:, ps],
                            GT[:].rearrange("p (a b) -> p a b", b=PEDGES),
                            axis=X, op=Op.mult)

                # chunk-wide: r^2 (split between ACT and GpSimd to balance
                # engine load); d2 = l^2 + r^2 on GpSimd; first-level min on
                # the DVE (u/uS/l columns are half-permuted so col i pairs
                # with col i + E/2), then the final 16-wide min reduce.
                rsq = apool.tile([128, E], bf16, tag="rsq")
                nc.scalar.activation(rsq[:, :E // 2], rall[:, :E // 2],
                                     Act.Square)
                nc.gpsimd.tensor_tensor(rsq[:, E // 2:], rall[:, E // 2:],
                                        rall[:, E // 2:], op=Op.mult)
                d2 = apool.tile([128, E], bf16, tag="d2")
                nc.vector.tensor_tensor(d2[:, :E // 2], lsq[:, :E // 2],
                                        rsq[:, :E // 2], op=Op.add)
                nc.gpsimd.tensor_tensor(d2[:, E // 2:], lsq[:, E // 2:],
                                        rsq[:, E // 2:], op=Op.add)
                m16 = apool.tile([128, E // 2], bf16, tag="m16")
                nc.vector.tensor_tensor(m16[:], d2[:, :E // 2], d2[:, E // 2:],
                                        op=Op.min)
                nc.vector.tensor_reduce(
                    mind2,
                    m16[:].rearrange("p (a b) -> p a b", b=PEDGES // 2),
                    axis=X, op=Op.min)

            # q = mind2 * sign(gprod) for ALL chunks at once: gprod's sign
            # is the crossing parity (one ACT + one DVE op instead of 9+9)
            sgnbig = cpool.tile([128, CHUNKS, NPT], f32)
            nc.scalar.activation(sgnbig[:], gprodbig[:], Act.Sign)
            nc.vector.tensor_tensor(qbig[:], mindbig[:], sgnbig[:],
                                    op=Op.mult)

            # per-batch min over polys -> qall[:, 32b + c]
            qall = cpool.tile([128, B * 32], f32)
            nc.gpsimd.memset(qall[:], 0)
            for b in range(B):
                nc.vector.tensor_reduce(
                    qall[:, 32 * b:32 * b + CHUNKS],
                    qbig[:, :, offs[b]:offs[b + 1]], axis=X, op=Op.min)

            # end stage: sigmoid(-SHARP * sign(q) * sqrt(|q|)) via exp/ln:
            #   rt = exp(ln(|q|)/2) = sqrt(|q|), capped at 0.5 (=> |x|<=50)
            #   out = 1 / (1 + exp(SHARP * sign(q) * rt))
            absq = wpool.tile([128, B * 32], f32, tag="absq")
            nc.scalar.activation(absq[:], qall[:], Act.Abs)
            absq2 = wpool.tile([128, B * 32], f32, tag="absq2")
            nc.vector.tensor_scalar(absq2[:], absq[:], 1e-30, None, op0=Op.add)
            lnq = wpool.tile([128, B * 32], f32, tag="lnq")
            nc.scalar.activation(lnq[:], absq2[:], Act.Ln)
            rt = wpool.tile([128, B * 32], f32, tag="rt")
            nc.scalar.activation(rt[:], lnq[:], Act.Exp, scale=0.5)
            sgq = wpool.tile([128, B * 32], f32, tag="sgq")
            nc.scalar.activation(sgq[:], qall[:], Act.Sign)
            w_ = wpool.tile([128, B * 32], f32, tag="w")
            nc.vector.scalar_tensor_tensor(
                w_[:], rt[:], 0.5, sgq[:], op0=Op.min, op1=Op.mult)
            ex_ = wpool.tile([128, B * 32], f32, tag="ex")
            nc.scalar.activation(ex_[:], w_[:], Act.Exp, scale=SHARP)
            s1 = wpool.tile([128, B * 32], f32, tag="s1")
            nc.vector.tensor_scalar(s1[:], ex_[:], 1.0, None, op0=Op.add)
            cpb = wpool.tile([128, B * 32], f32, tag="cpb")
            nc.vector.reciprocal(cpb[:], s1[:])

            pst = opool.tile([128, 128], f32, tag="pp", name="pst")
            nc.tensor.transpose(pst[:], cpb[:], ident[:])
            comb = cpool.tile([128, 128], f32, tag="comb", name="comb")
            nc.scalar.activation(comb[:], pst[:], Act.Copy)

            # the device output is the [B, MP] soft mask; the host replicates
            # it across the depth axis (pure data tiling) during unshard.
            engs = [nc.sync, nc.gpsimd, nc.scalar]
            for b in range(B):
                engs[b % 3].dma_start(out_d[b:b + 1, :],
                                      comb[32 * b:32 * b + CHUNKS, :])

    nc.compile()
    return nc


def _enable_ldw_opt():
    # no-op: --enable-ldw-opt=true breaks walrus codegen on the transpose's
    # standalone InstLdweights (kept so callers can still invoke it).
    return


def kernel(polygons, attributes, validity_scores):
    from concourse.bass_utils import run_bass_kernel_spmd

    B = polygons.shape[0]
    tables, Ws, feats, counts, E, hvs = _host_prep(
        polygons, attributes, validity_scores)
    nc = _build(B, counts, E, hvs)
    in_maps = [dict(tables, W=Ws[k], feat=feats[k]) for k in range(NCORES)]
    res = run_bass_kernel_spmd(nc, in_maps, list(range(NCORES))).results
    parts = [res[k]["out"].reshape(B, VOX // NCORES, VOX)
             for k in range(NCORES)]
    combined = np.concatenate(parts, axis=1)          # [B, VOX, VOX]
    dmask = (np.arange(VOX, dtype=np.float32)[None, :]
             < np.array(hvs, np.float32)[:, None]).astype(np.float32)
    voxels = combined[:, None, :, :] * dmask[:, :, None, None]
    return np.ascontiguousarray(voxels, np.float32)
